# revision 1
# baseline (speedup 1.0000x reference)
"""Trainium2 Bass kernel for nn_CausalSelfAttention_38216619000057.

Reference semantics (faithful to the source bug q = k):
    qkv = x @ W_attn + b_attn ; _, k, v = split(qkv)
    S = (K K^T) * D**-0.5  (per head, causal-masked), P = softmax(S)
    out = (P V) reshaped @ W_proj + b_proj

Sharding over 8 cores: data-parallel on B (4), tensor-parallel on heads (2
groups of 8). Core c handles batch c//2, heads 8*(c%2)..8*(c%2)+7, and
produces a partial projection output; the host sums the two partials per
batch and adds b_proj + b_v @ W_proj (the V-bias contribution commutes
through softmax because rows of P sum to 1).

Precision / engine strategy (PE is the bottleneck; ~2.5x faster than the
f32r baseline):
  * The attention-logit path runs in fp8e4m3 with DoubleRow perf mode (2
    moving rows/cycle, 157 TF/s): the K projection (contraction pairs over
    embedding halves) and S^T = K K^T (contraction pairs over hd halves,
    heads at partition bases {0,32} of two kt8 tiles -- matmul operands may
    only start at partitions 0/32/64, so the upper head-groups are shifted
    down once per chunk by a cheap SBUF->SBUF DMA). Softmax normalization +
    output averaging tame the fp8 quantization noise (~0.7% final error,
    budget 2e-2).
  * PV runs fp8 DoubleRow on the far (never-masked) key-block pairs -- 60%
    of the block area but only ~30% of the attention mass, so the eps
    quantization adds just ~0.7% error (verified end-to-end in numpy, which
    predicts the device error to 4 digits). The V operand's fp8 error is
    compensated exactly: V = fp8(V) + fp8(V - fp8(V)), two DR matmuls (still
    2x fewer PE cycles than one bf16 matmul). DoubleRow output partition
    counts must be in {64, 96, 128} -- the 64 V-cols + ones-column are
    padded to 96 with zero weight columns (pv rows 65:96 are never read).
  * Near-diagonal pairs (most of the attention mass) and everything whose
    error lands directly in the output stay >= bf16: x^T/V-weights bf16,
    crossing-pair PV bf16, output projection bf16, PSUM f32.
  * x is transposed on the HOST and uploaded twice (fp8 pair-layout for the
    K matmul, bf16 for V) -- no on-device transposes at all.
  * Causal mask: GPSIMD affine_select zeroes masked entries of the exp
    tiles on the diagonal-crossing blocks (no PE mask matmuls, no -inf
    logits); for the 2nd crossing pair of each chunk, the all-masked query
    range [0,256) is skipped on PE and Act entirely.
  * Softmax denominators come free via a ones-column appended to V; the
    reciprocal row is broadcast across partitions on GPSIMD; PSUM->SBUF
    copies run on the Activation engine (3x faster on real HW than the
    cost model claims; exp is nowhere near the bottleneck there).

Work is software-pipelined per 512-query chunk: the K/V projection pieces
of chunk ci+1 and the output-projection pieces of chunk ci-1 are emitted as
fillers inside and between the attention head-pieces of chunk ci (weighted
toward the back half of the chunk), so PE keeps executing through the
exp/normalization latency at head and chunk boundaries. Input DMAs issue
from three engine queues with the K-path dependencies first.
"""

import threading

import numpy as np

import concourse.bacc as bacc
import concourse.mybir as mybir
import concourse.tile as tile
from concourse.bass_utils import run_bass_kernel_spmd

B, T, D = 4, 2048, 1024
H = 16
HD = 64
NCORES = 8
HPC = 8  # heads per core
ISQ = float(D**-0.5) ** 0.5  # K is pre-scaled by sqrt(D**-0.5)
WK_SCALE = 64.0  # keeps fp8 W_k columns in e4m3 normal range
F32 = mybir.dt.float32
F32R = mybir.dt.float32r
BF16 = mybir.dt.bfloat16
F8 = mybir.dt.float8e4
NEG = -224.0  # e4m3 max normal is 240
DR = mybir.MatmulPerfMode.DoubleRow

Ident = mybir.ActivationFunctionType.Identity
Exp = mybir.ActivationFunctionType.Exp
Mult = mybir.AluOpType.mult

_cache_lock = threading.Lock()
_cached_nc = {}


def _declare_io(nc, synth=False):
    kind = "Internal" if synth else "ExternalInput"
    ts = {}
    # x^T in fp8 pair layout for the DoubleRow K matmul:
    # x8[p, ei, eb, t] = x[t, 256*eb + 128*ei + p]
    ts["x8"] = nc.dram_tensor("x8", [128, 2, 4, T], F8, kind=kind)
    # fp8 residual of x^T (same pair layout) for the compensated V matmul
    ts["x8l"] = nc.dram_tensor("x8l", [128, 2, 4, T], F8, kind=kind)
    # W_k fp8 (x WK_SCALE), permuted so PSUM partitions land in kt8 layout:
    # wk[p, ei, eb, u, j] with u=(m,di), j=32a+d -> k-col (4m+a)*64+32*di+d
    ts["wk"] = nc.dram_tensor("wk", [128, 2, 4, 4, 128], F8, kind=kind)
    # W_v x 64 as fp8 hi + residual, in the x8 e-layout (e=256eb+128ei+p)
    ts["wvh"] = nc.dram_tensor("wvh", [128, 2, 4, 512], F8, kind=kind)
    ts["wvl"] = nc.dram_tensor("wvl", [128, 2, 4, 512], F8, kind=kind)
    # W_proj x64 as fp8 hi + residual; pair dim = hp parity (hpi), so the
    # DR projection contracts 256/step: wp8[64q+d, hpi, hpp, n]
    ts["wph"] = nc.dram_tensor("wph", [128, 2, 2, 1024], F8, kind=kind)
    ts["wpl"] = nc.dram_tensor("wpl", [128, 2, 2, 1024], F8, kind=kind)
    ts["bk"] = nc.dram_tensor("bk", [128, 4], F32, kind=kind)
    ts["out"] = nc.dram_tensor("out", [T, D], F32, kind="Internal" if synth else "ExternalOutput")
    if synth:
        ts["done"] = nc.dram_tensor("done", [1, 4], F32, kind="ExternalOutput")
    return ts


def _synth_init(nc, tc, io):
    """Fill the Internal input tensors with benign constants on device."""
    with tc.tile_pool(name="init", bufs=1) as pool:
        zt = pool.tile([128, 8192], F32, name="init_t")
        nc.vector.memset(zt[:], 0.0)
        nc.sync.dma_start(
            io["x8"][:],
            zt[:, 0:4096].bitcast(F8).rearrange("p (i e t) -> p i e t", i=2, e=4),
        )
        nc.sync.dma_start(
            io["x8l"][:],
            zt[:, 0:4096].bitcast(F8).rearrange("p (i e t) -> p i e t", i=2, e=4),
        )
        nc.sync.dma_start(
            io["wk"][:],
            zt[:, 0:1024].bitcast(F8).rearrange("p (i e u j) -> p i e u j", i=2, e=4, u=4),
        )
        nc.sync.dma_start(
            io["wvh"][:],
            zt[:, 0:1024].bitcast(F8).rearrange("p (i e n) -> p i e n", i=2, e=4),
        )
        nc.sync.dma_start(
            io["wvl"][:],
            zt[:, 0:1024].bitcast(F8).rearrange("p (i e n) -> p i e n", i=2, e=4),
        )
        nc.sync.dma_start(
            io["wph"][:],
            zt[:, 0:1024].bitcast(F8).rearrange("p (i h n) -> p i h n", i=2, h=2),
        )
        nc.sync.dma_start(
            io["wpl"][:],
            zt[:, 0:1024].bitcast(F8).rearrange("p (i h n) -> p i h n", i=2, h=2),
        )
        nc.sync.dma_start(io["bk"][:], zt[:, 0:4])


def _emit_body(nc, tc, io, g):
    """One full forward pass. g holds the persistent SBUF tiles."""
    kt8, v_ones = g["kt8"], g["v_ones"]
    o8b, o8l = g["o8b"], g["o8l"]
    kt8b = g["kt8b"]
    v8h, v8l = g["v8h"], g["v8l"]
    x8_sb, x8l_sb = g["x8_sb"], g["x8l_sb"]
    wk_sb, wvh_sb, wvl_sb = g["wk_sb"], g["wvh_sb"], g["wvl_sb"]
    wph_sb, wpl_sb = g["wph_sb"], g["wpl_sb"]
    bk_sb = g["bk_sb"]
    out = io["out"]

    with (
        tc.tile_pool(name="ps_s", bufs=2, space="PSUM") as ps_s,
        tc.tile_pool(name="ps_pv", bufs=2, space="PSUM") as ps_pv,
        tc.tile_pool(name="ps1", bufs=2, space="PSUM") as ps1,
        tc.tile_pool(name="ebuf", bufs=6) as ebuf,
        tc.tile_pool(name="rbuf", bufs=4) as rbuf,
        tc.tile_pool(name="obuf", bufs=4) as obuf,
    ):

        def a_pieces(ci):
            """K and V projection pieces for t-chunk ci (8 pieces)."""
            ps = []
            cs = slice(ci * 512, ci * 512 + 512)

            def k_piece(u, cs=cs, ci=ci):
                kps = ps1.tile([128, 512], F32, tag="ps1", name="kps")
                for eb in range(4):
                    nc.tensor.matmul(
                        kps[:],
                        wk_sb[:, :, eb, u, :],
                        x8_sb[:, :, eb, cs],
                        start=(eb == 0),
                        stop=(eb == 3),
                        perf_mode=DR,
                    )
                m, di = u // 2, u % 2
                # bias+scale on Act (3x faster than modeled on real HW;
                # DVE is the tighter engine after the fp8-DR conversions)
                nc.scalar.activation(
                    kt8[:, di, m, cs],
                    kps[:],
                    Ident,
                    bias=bk_sb[:, u : u + 1],
                    scale=ISQ / WK_SCALE,
                )

            def v_piece(tbl, ci=ci):
                vps = ps1.tile([128, 512], F32, tag="ps1", name="vps")
                tb = 4 * ci + tbl
                tbs = slice(tb * 128, tb * 128 + 128)
                # compensated fp8 DoubleRow: 64*V = x8(wvh+wvl) + x8lo*wvh
                # (the 64x scale cancels in the softmax normalization)
                terms = [(x8_sb, wvh_sb), (x8_sb, wvl_sb), (x8l_sb, wvh_sb)]
                for eb in range(4):
                    for ti, (xs, ws) in enumerate(terms):
                        nc.tensor.matmul(
                            vps[:],
                            xs[:, :, eb, tbs],
                            ws[:, :, eb, :],
                            start=(eb == 0 and ti == 0),
                            stop=(eb == 3 and ti == 2),
                            perf_mode=DR,
                        )
                nc.scalar.copy(
                    v_ones[:, tb, :].rearrange("p (h c) -> p h c", c=65)[:, :, 0:64],
                    vps[:].rearrange("p (h c) -> p h c", c=64),
                )
                # fp8 hi + residual copies for the DoubleRow PV on far pairs
                h_sl = v8h[:, tb // 2, tb % 2, :].rearrange(
                    "p (h c) -> p h c", c=96)[:, :, 0:64]
                nc.scalar.copy(h_sl, vps[:].rearrange("p (h c) -> p h c", c=64))
                nc.vector.tensor_tensor(
                    v8l[:, tb // 2, tb % 2, :].rearrange(
                        "p (h c) -> p h c", c=96)[:, :, 0:64],
                    vps[:].rearrange("p (h c) -> p h c", c=64),
                    h_sl,
                    mybir.AluOpType.subtract,
                )

            def kshift_piece(cs=cs):
                # matmul operands may only start at partition 0/32/64; shift
                # the upper head-groups down so S reads at bases {0, 32}
                nc.sync.dma_start(kt8b[:, :, :, cs], kt8[64:128, :, :, cs])

            for u in range(4):
                ps.append(lambda u=u: k_piece(u))
            ps.append(kshift_piece)
            for tbl in range(4):
                ps.append(lambda tbl=tbl: v_piece(tbl))
            return ps

        def b_head_piece(ci, l, filler=None):
            """Attention for local head l (=4m+a) over query chunk ci.

            `filler` (if given) is emitted late in the pair loop, where PE
            otherwise stalls on sps buffers waiting for Act to drain exps.
            """
            m, a = l // 4, l % 4
            src = kt8 if a < 2 else kt8b
            pr = slice(32 * (a % 2), 32 * (a % 2) + 32)
            cs = slice(ci * 512, ci * 512 + 512)
            njb = 4 * ci + 4
            pv = ps_pv.tile([96, 512], F32, tag="pv", name="pv")
            r_row = rbuf.tile([1, 512], F32, tag="rr", name="r_row")
            r_bc = rbuf.tile([64, 512], F32, tag="rb", name="r_bc")
            of = rbuf.tile([128, 512], F32, tag="of", name="of")
            q_, hp = l // 4, l % 4
            hpi, hpp = hp % 2, hp // 2
            po = slice(64 * q_, 64 * q_ + 64)

            def norm_half(qa, qb):
                # normalize finished queries while PE continues; the ones
                # columns hold 4.0 (=64/16) so `of` comes out as 16*o --
                # fp8's normal range for both the hi part and the residual
                qo = slice(ci * 512 + qa, ci * 512 + qb)
                nc.vector.reciprocal(r_row[:, qa:qb], pv[64:65, qa:qb])
                nc.gpsimd.partition_broadcast(r_bc[:, qa:qb], r_row[:, qa:qb])
                nc.vector.tensor_tensor(
                    of[po, qa:qb], pv[0:64, qa:qb], r_bc[:, qa:qb], Mult
                )
                nc.scalar.copy(o8b[po, hpi, hpp, qo], of[po, qa:qb])
                nc.gpsimd.tensor_tensor(
                    o8l[po, hpi, hpp, qo],
                    of[po, qa:qb],
                    o8b[po, hpi, hpp, qo],
                    mybir.AluOpType.subtract,
                )

            fill_at = max(njb // 2 - 2, 0)
            for jbp in range(njb // 2):
                if jbp == fill_at and filler is not None:
                    filler()
                # for the 2nd diagonal-crossing pair, queries [0, 256) are
                # entirely masked -- skip them on PE and Act
                q0 = 256 * (jbp - 2 * ci) if jbp >= 2 * ci else 0
                qs = slice(ci * 512 + q0, ci * 512 + 512)
                sps = ps_s.tile([128, 2, 512], F32, tag="s", name="sps")
                for half in (0, 1):
                    jb = 2 * jbp + half
                    nc.tensor.matmul(
                        sps[:, half, q0:512],
                        src[pr, :, m, jb * 128 : (jb + 1) * 128],
                        src[pr, :, m, qs],
                        start=True,
                        stop=True,
                        perf_mode=DR,
                    )
                if jbp < 2 * ci:
                    # far (never-masked) pair: fp8 eps + DoubleRow PV over
                    # the block pair, V error compensated by an fp8 residual
                    # term -- 2x fewer PE cycles than the bf16 path
                    ep8 = ebuf.tile([128, 2, 512], F8, tag="e8", name="ep8")
                    nc.scalar.activation(ep8[:], sps[:], Exp)
                    for vi, vt in enumerate((v8h, v8l)):
                        nc.tensor.matmul(
                            pv[:],
                            vt[:, jbp, :, 96 * l : 96 * l + 96],
                            ep8[:],
                            start=(jbp == 0 and vi == 0),
                            stop=False,
                            perf_mode=DR,
                            skip_group_check=True,
                        )
                    continue
                eps = ebuf.tile([128, 2, 512], BF16, tag="e", name="eps")
                nc.scalar.activation(eps[:, :, q0:512], sps[:, :, q0:512], Exp)
                # diagonal-crossing pair: zero masked entries (j > q) on
                # the idle GPSIMD engine instead of a PE mask matmul
                epm = ebuf.tile([128, 2, 512], BF16, tag="em", name="epm")
                for half in (0, 1):
                    oi = 2 * jbp + half - 4 * ci
                    nc.gpsimd.affine_select(
                        epm[:, half, q0:512],
                        eps[:, half, q0:512],
                        pattern=[[1, 512 - q0]],
                        compare_op=mybir.AluOpType.is_ge,
                        fill=0.0,
                        base=q0 - 128 * oi,
                        channel_multiplier=-1,
                    )
                eps = epm
                for half in (0, 1):
                    jb = 2 * jbp + half
                    nc.tensor.matmul(
                        pv[0:65, q0:512],
                        v_ones[:, jb, 65 * l : 65 * l + 65],
                        eps[:, half, q0:512],
                        start=(jb == 0 and ci == 0),
                        stop=(jb == njb - 1) or (jb == 4 * ci + 1),
                        skip_group_check=True,
                    )
                if jbp == 2 * ci:
                    norm_half(0, 256)
            norm_half(256, 512)

        def proj_pieces(ci):
            """Output projection pieces for t-chunk ci (8 pieces)."""
            ps = []

            def p_piece(tbl, nch, ci=ci):
                tb = 4 * ci + tbl
                tbs = slice(tb * 128, tb * 128 + 128)
                ns = slice(nch * 512, nch * 512 + 512)
                ops_ = ps1.tile([128, 512], F32, tag="ps1", name="ops")
                terms = [(o8b, wph_sb), (o8b, wpl_sb), (o8l, wph_sb)]
                for hpp2 in range(2):
                    for ti, (os_, ws_) in enumerate(terms):
                        nc.tensor.matmul(
                            ops_[:],
                            os_[:, :, hpp2, tbs],
                            ws_[:, :, hpp2, ns],
                            start=(hpp2 == 0 and ti == 0),
                            stop=(hpp2 == 1 and ti == 2),
                            perf_mode=DR,
                        )
                ob = obuf.tile([128, 512], F32, tag="ob", name="ob")
                nc.scalar.mul(ob[:], ops_[:], 1.0 / 1024.0)
                nc.sync.dma_start(
                    out[tb * 128 : (tb + 1) * 128, nch * 512 : (nch + 1) * 512],
                    ob[:],
                )

            for tbl in range(4):
                for nch in range(2):
                    ps.append(lambda tbl=tbl, nch=nch: p_piece(tbl, nch))
            return ps

        for f in a_pieces(0):
            f()
        for ci in range(4):
            fillers = (a_pieces(ci + 1) if ci < 3 else []) + (
                proj_pieces(ci - 1) if ci > 0 else []
            )
            # one filler inside each head (covers the mid-head sps-buffer
            # wall); spread the rest across the BACK half of the chunk so the
            # chunk tail (normalization chains) stays covered
            nrest = max(len(fillers) - 8, 0)
            bounds = [8 + (nrest * k) // 5 for k in range(6)]
            for i in range(8):
                b_head_piece(ci, i, fillers[i] if i < len(fillers) else None)
                if i >= 3:
                    for f in fillers[bounds[i - 3] : bounds[i - 2]]:
                        f()
            for f in fillers[bounds[5] :]:
                f()
        for f in proj_pieces(3):
            f()


def _build_program(nreps: int = 1, synth: bool = False):
    nc = bacc.Bacc("TRN2", target_bir_lowering=False)
    io = _declare_io(nc, synth=synth)

    with tile.TileContext(nc) as tc:
        if synth:
            _synth_init(nc, tc, io)
        with tc.tile_pool(name="singles", bufs=1) as singles:
            g = {}
            g["kt8"] = singles.tile([128, 2, 2, T], F8, name="kt8")
            g["kt8b"] = singles.tile([64, 2, 2, T], F8, name="kt8b")
            g["v_ones"] = singles.tile([128, 16, HPC * 65], BF16, name="v_ones")
            g["v8h"] = singles.tile([128, 8, 2, HPC * 96], F8, name="v8h")
            g["v8l"] = singles.tile([128, 8, 2, HPC * 96], F8, name="v8l")
            g["o8b"] = singles.tile([128, 2, 2, T], F8, name="o8b")
            g["o8l"] = singles.tile([128, 2, 2, T], F8, name="o8l")
            g["x8_sb"] = singles.tile([128, 2, 4, T], F8, name="x8_sb")
            g["x8l_sb"] = singles.tile([128, 2, 4, T], F8, name="x8l_sb")
            g["wk_sb"] = singles.tile([128, 2, 4, 4, 128], F8, name="wk_sb")
            g["wvh_sb"] = singles.tile([128, 2, 4, 512], F8, name="wvh_sb")
            g["wvl_sb"] = singles.tile([128, 2, 4, 512], F8, name="wvl_sb")
            g["wph_sb"] = singles.tile([128, 2, 2, 1024], F8, name="wph_sb")
            g["wpl_sb"] = singles.tile([128, 2, 2, 1024], F8, name="wpl_sb")
            g["bk_sb"] = singles.tile([128, 4], F32, name="bk_sb")

            # pad-column init first so Pool's in-order queue never delays
            # the causal selects behind its DMA descriptor generation
            nc.gpsimd.memset(
                g["v8h"][:]
                .rearrange("p j i (h c) -> p j i h c", c=96)[:, :, :, :, 64:96],
                0.0,
            )
            nc.gpsimd.memset(
                g["v8h"][:]
                .rearrange("p j i (h c) -> p j i h c", c=96)[:, :, :, :, 64:65],
                4.0,
            )
            nc.vector.memset(
                g["v8l"][:]
                .rearrange("p j i (h c) -> p j i h c", c=96)[:, :, :, :, 64:96],
                0.0,
            )
            # K-path deps (wk, bk, x8 chunk 0) first on the sync queue so the
            # first matmul starts ASAP; bulk xb/wv/wp issue from other engine
            # queues to overlap.
            nc.sync.dma_start(g["wk_sb"][:], io["wk"][:])
            nc.scalar.dma_start(g["wvh_sb"][:], io["wvh"][:])
            nc.scalar.dma_start(g["wvl_sb"][:], io["wvl"][:])
            nc.scalar.dma_start(g["wph_sb"][:], io["wph"][:])
            nc.scalar.dma_start(g["wpl_sb"][:], io["wpl"][:])
            nc.gpsimd.dma_start(g["bk_sb"][:], io["bk"][:])
            for ci in range(4):
                cs = slice(ci * 512, ci * 512 + 512)
                nc.sync.dma_start(g["x8_sb"][:, :, :, cs], io["x8"][:, :, :, cs])
                nc.gpsimd.dma_start(g["x8l_sb"][:, :, :, cs], io["x8l"][:, :, :, cs])
            nc.vector.memset(
                g["v_ones"][:]
                .rearrange("p t (h c) -> p t h c", c=65)[:, :, :, 64:65],
                4.0,
            )


            for _rep in range(nreps):
                _emit_body(nc, tc, io, g)

            if synth:
                with tc.tile_pool(name="fin", bufs=1) as fin:
                    dn = fin.tile([1, 4], F32, name="dn")
                    nc.vector.memset(dn[:], 1.0)
                    nc.sync.dma_start(io["done"][:], dn[:])

    nc.compile()
    return nc


def _build_null_program():
    """Same I/O signature, trivial body -- for wall-clock differencing."""
    nc = bacc.Bacc("TRN2", target_bir_lowering=False)
    io = _declare_io(nc)
    with tile.TileContext(nc) as tc:
        with tc.tile_pool(name="sb", bufs=2) as sb:
            t = sb.tile([128, 512], F32)
            nc.sync.dma_start(t[:].bitcast(F8), io["x8"][:, 0, 0, 0:2048])
            for tb in range(16):
                for nch in range(2):
                    nc.sync.dma_start(
                        io["out"][
                            tb * 128 : (tb + 1) * 128, nch * 512 : (nch + 1) * 512
                        ],
                        t[:],
                    )
    nc.compile()
    return nc


def _get_program(nreps: int = 1, synth: bool = False):
    with _cache_lock:
        key = (nreps, synth)
        if key not in _cached_nc:
            _cached_nc[key] = _build_program(nreps, synth)
        return _cached_nc[key]


def _core_inputs(c, x, W_attn, b_attn, a_np, b_np):
    import ml_dtypes

    f8 = ml_dtypes.float8_e4m3
    bf16 = ml_dtypes.bfloat16
    b = c // 2
    h0 = HPC * (c % 2)
    c0k = D + h0 * HD
    c0v = 2 * D + h0 * HD
    xt = np.ascontiguousarray(x[b].T)  # [D, T]
    # x8[p, ei, eb, t] = xt[256*eb + 128*ei + p, t]
    x8 = np.ascontiguousarray(
        xt.reshape(4, 2, 128, T).transpose(2, 1, 0, 3).astype(f8)
    )
    # fp8 residual of x^T in the same pair layout
    x8l = np.ascontiguousarray(
        (xt.reshape(4, 2, 128, T).transpose(2, 1, 0, 3)
         - x8.astype(np.float32)).astype(f8)
    )
    # wk[p, ei, eb, u, j=32a+d] = W[256eb+128ei+p, c0k + (4m+a)*64+32di+d]*WK_SCALE
    wkc = W_attn[:, c0k : c0k + 512] * WK_SCALE  # [1024, 512]
    # k-col layout: (m, a, di, d) -> col (4m+a)*64 + 32di + d
    wkc = wkc.reshape(4, 2, 128, 2, 4, 2, 32)  # [eb, ei, p, m, a, di, d]
    wk = np.ascontiguousarray(
        wkc.transpose(2, 1, 0, 3, 5, 4, 6)  # [p, ei, eb, m, di, a, d]
        .reshape(128, 2, 4, 4, 128)
        .astype(f8)
    )
    # W_v x64 (fp8 normal range) split hi + residual, x8 e-layout
    wvc = (W_attn[:, c0v : c0v + 512] * 64.0).reshape(4, 2, 128, 512).transpose(2, 1, 0, 3)
    wvh = np.ascontiguousarray(wvc.astype(f8))
    wvl = np.ascontiguousarray((wvc - wvh.astype(np.float32)).astype(f8))
    # bk[p=32a+d, u=(m,di)] = b_attn[c0k + (4m+a)*64 + 32di + d] * ISQ
    bkc = b_attn[c0k : c0k + 512].reshape(2, 4, 2, 32)  # [m, a, di, d]
    bk = np.ascontiguousarray(
        (bkc.transpose(1, 3, 0, 2) * ISQ)  # [a, d, m, di]
        .reshape(128, 4)
        .astype(np.float32)
    )
    return {
        "x8": x8,
        "x8l": x8l,
        "wk": wk,
        "wvh": wvh,
        "wvl": wvl,
        "bk": bk,
    }


def _core_wp(c, W_proj):
    import ml_dtypes

    f8 = ml_dtypes.float8_e4m3
    h0 = HPC * (c % 2)
    r0 = h0 * HD
    # wp8[p=64q+d, hpi, hpp, n] = W_proj[r0 + (4q+2hpp+hpi)*64 + d, n] * 64
    wpc = (
        (W_proj[r0 : r0 + 512, :] * 64.0)
        .reshape(2, 2, 2, 64, 1024)
        .transpose(0, 3, 2, 1, 4)
        .reshape(128, 2, 2, 1024)
    )
    wph = np.ascontiguousarray(wpc.astype(f8))
    wpl = np.ascontiguousarray((wpc - wph.astype(np.float32)).astype(f8))
    return {"wph": wph, "wpl": wpl}


def _mask_mats():
    import ml_dtypes

    f8 = ml_dtypes.float8_e4m3
    P = np.arange(128)
    an = np.where(P[:, None] <= P[None, :], np.float32(NEG), np.float32(0.0))
    il = np.arange(512)
    bo = np.zeros((128, 4, 512), dtype=np.float32)
    for oi in range(4):
        bo[:, oi, :] = (il[None, :] < (P[:, None] + 128 * oi)).astype(np.float32)
    # split contraction dim 128 -> [64, 2] for DoubleRow (P = 64*i2 + p')
    a_np = np.ascontiguousarray(an.reshape(2, 64, 128).transpose(1, 0, 2).astype(f8))
    b_np = np.ascontiguousarray(
        bo.reshape(2, 64, 4, 512).transpose(1, 0, 2, 3).astype(f8)
    )
    return a_np, b_np


def kernel(x, W_attn, b_attn, W_proj, b_proj, **_unused):
    x = np.asarray(x, dtype=np.float32)
    W_attn = np.asarray(W_attn, dtype=np.float32)
    b_attn = np.asarray(b_attn, dtype=np.float32)
    W_proj = np.asarray(W_proj, dtype=np.float32)
    b_proj = np.asarray(b_proj, dtype=np.float32)

    nc = _get_program()
    a_np, b_np = _mask_mats()
    in_maps = []
    for c in range(NCORES):
        m = _core_inputs(c, x, W_attn, b_attn, a_np, b_np)
        m.update(_core_wp(c, W_proj))
        in_maps.append(m)

    res = run_bass_kernel_spmd(nc, in_maps, core_ids=list(range(NCORES)))

    bias_row = b_proj + b_attn[2 * D : 3 * D] @ W_proj
    out = np.empty((B, T, D), dtype=np.float32)
    for b in range(B):
        out[b] = res.results[2 * b]["out"] + res.results[2 * b + 1]["out"] + bias_row
    return out



# revision 2
# speedup vs baseline: 3.5394x; 3.5394x over previous
"""Trainium2 Bass kernel for nn_CausalSelfAttention_38216619000057.

Reference semantics (faithful to the source bug q = k):
    qkv = x @ W_attn + b_attn ; _, k, v = split(qkv)
    S = (K K^T) * D**-0.5  (per head, causal-masked), P = softmax(S)
    out = (P V) reshaped @ W_proj + b_proj

Sharding over 8 cores: data-parallel on B (4), tensor-parallel on heads (2
groups of 8). Core c handles batch c//2, heads 8*(c%2)..8*(c%2)+7, and
produces a partial projection output; the host sums the two partials per
batch and adds b_proj + b_v @ W_proj (the V-bias contribution commutes
through softmax because rows of P sum to 1).

Engine strategy (all four compute engines balanced at ~125us of modeled
work; the f32r baseline was Act-bound at ~228us):
  * Attention-logit path in fp8e4m3 with DoubleRow perf mode (2 moving
    rows/cycle): K projection and S^T = K K^T as before.
  * The exp of the never-masked far key-block pairs is split between the
    Activation engine (true exp -> fp8, 2/3 of pairs) and the DVE engine
    (1/3 of pairs) using a one-instruction Schraudolph fast-exp: fp8 bits
    B = round(8*log2(e)*s + 55.65) computed by tensor_scalar with uint8
    saturating output, bitcast to fp8 -- the float->uint8 convert rounds
    to nearest and clamps at 0 (deep-negative logits become +0.0).
  * PV on far pairs stays fp8-DR with the exactly-compensated V =
    fp8(V) + fp8(V - fp8(V)) pair of matmuls.
  * Near-diagonal (crossing) pairs keep true exp -> bf16 on Act, causal
    zeroing by GPSIMD affine_select, and bf16 PV matmuls.
  * The attention output `o` is stored bf16 (not compensated fp8): the
    normalization multiply on DVE writes bf16 directly, and the output
    projection runs as 4 bf16 matmuls per tile. This deletes the o8
    hi/lo copies that used to occupy Act and Pool.
  * K-projection bias+scale, the fp8 V hi-copy, and the PSUM->out copy
    moved from Act to DVE tensor_scalar ops; output DMA is bf16 (host
    upcasts and adds the bias row).
  * Softmax denominators come free via a ones-column appended to V; the
    reciprocal row is broadcast across partitions on GPSIMD.

Work is software-pipelined per 512-query chunk: the K/V projection pieces
of chunk ci+1 and the output-projection pieces of chunk ci-1 are emitted as
fillers inside and between the attention head-pieces of chunk ci (weighted
toward the back half of the chunk), so PE keeps executing through the
exp/normalization latency at head and chunk boundaries. Input DMAs issue
from three engine queues with the K-path dependencies first.
"""

import threading

import numpy as np

import concourse.bacc as bacc
import concourse.mybir as mybir
import concourse.tile as tile
from concourse.bass_utils import run_bass_kernel_spmd

B, T, D = 4, 2048, 1024
H = 16
HD = 64
NCORES = 8
HPC = 8  # heads per core
ISQ = float(D**-0.5) ** 0.5  # K is pre-scaled by sqrt(D**-0.5)
WK_SCALE = 64.0  # keeps fp8 W_k columns in e4m3 normal range
LOG2E = 1.4426950408889634
FE_A = 8.0 * LOG2E  # fast-exp slope (fp8 bits per nat)
FE_B = 55.65  # fast-exp magic bias (56 - Schraudolph shift)
F32 = mybir.dt.float32
F32R = mybir.dt.float32r
BF16 = mybir.dt.bfloat16
F8 = mybir.dt.float8e4
U8 = mybir.dt.uint8
DR = mybir.MatmulPerfMode.DoubleRow

Ident = mybir.ActivationFunctionType.Identity
Exp = mybir.ActivationFunctionType.Exp
Mult = mybir.AluOpType.mult
Add = mybir.AluOpType.add

_cache_lock = threading.Lock()
_cached_nc = {}


def _declare_io(nc, synth=False):
    kind = "Internal" if synth else "ExternalInput"
    ts = {}
    # x^T in fp8 pair layout for the DoubleRow K matmul:
    # x8[p, ei, eb, t] = x[t, 256*eb + 128*ei + p]
    ts["x8"] = nc.dram_tensor("x8", [128, 2, 4, T], F8, kind=kind)
    # fp8 residual of x^T (same pair layout) for the compensated V matmul
    ts["x8l"] = nc.dram_tensor("x8l", [128, 2, 4, T], F8, kind=kind)
    # W_k fp8 (x WK_SCALE), permuted so PSUM partitions land in kt8 layout:
    # wk[p, ei, eb, u, j] with u=(m,di), j=32a+d -> k-col (4m+a)*64+32*di+d
    ts["wk"] = nc.dram_tensor("wk", [128, 2, 4, 4, 128], F8, kind=kind)
    # W_v x 64 as fp8 hi + residual, in the x8 e-layout (e=256eb+128ei+p)
    ts["wvh"] = nc.dram_tensor("wvh", [128, 2, 4, 512], F8, kind=kind)
    ts["wvl"] = nc.dram_tensor("wvl", [128, 2, 4, 512], F8, kind=kind)
    # W_proj/16 bf16: wpb[p, blk, n] = W_proj[r0 + 128*blk + p, n] / 16
    ts["wpb"] = nc.dram_tensor("wpb", [128, 4, 1024], BF16, kind=kind)
    ts["bk"] = nc.dram_tensor("bk", [128, 4], F32, kind=kind)
    ts["out"] = nc.dram_tensor("out", [T, D], BF16, kind="Internal" if synth else "ExternalOutput")
    if synth:
        ts["done"] = nc.dram_tensor("done", [1, 4], F32, kind="ExternalOutput")
    return ts


def _synth_init(nc, tc, io):
    """Fill the Internal input tensors with benign constants on device."""
    with tc.tile_pool(name="init", bufs=1) as pool:
        zt = pool.tile([128, 8192], F32, name="init_t")
        nc.vector.memset(zt[:], 0.0)
        nc.sync.dma_start(
            io["x8"][:],
            zt[:, 0:4096].bitcast(F8).rearrange("p (i e t) -> p i e t", i=2, e=4),
        )
        nc.sync.dma_start(
            io["x8l"][:],
            zt[:, 0:4096].bitcast(F8).rearrange("p (i e t) -> p i e t", i=2, e=4),
        )
        nc.sync.dma_start(
            io["wk"][:],
            zt[:, 0:1024].bitcast(F8).rearrange("p (i e u j) -> p i e u j", i=2, e=4, u=4),
        )
        nc.sync.dma_start(
            io["wvh"][:],
            zt[:, 0:1024].bitcast(F8).rearrange("p (i e n) -> p i e n", i=2, e=4),
        )
        nc.sync.dma_start(
            io["wvl"][:],
            zt[:, 0:1024].bitcast(F8).rearrange("p (i e n) -> p i e n", i=2, e=4),
        )
        nc.sync.dma_start(
            io["wpb"][:],
            zt[:, 0:2048].bitcast(BF16).rearrange("p (b n) -> p b n", b=4),
        )
        nc.sync.dma_start(io["bk"][:], zt[:, 0:4])


def _emit_body(nc, tc, io, g):
    """One full forward pass. g holds the persistent SBUF tiles."""
    kt8, v_ones = g["kt8"], g["v_ones"]
    kt8b = g["kt8b"]
    v8h, v8l = g["v8h"], g["v8l"]
    o_bf = g["o_bf"]
    x8_sb, x8l_sb = g["x8_sb"], g["x8l_sb"]
    wk_sb, wvh_sb, wvl_sb = g["wk_sb"], g["wvh_sb"], g["wvl_sb"]
    wpb_sb = g["wpb_sb"]
    bk_sb = g["bk_sb"]
    out = io["out"]

    with (
        tc.tile_pool(name="ps_s", bufs=2, space="PSUM") as ps_s,
        tc.tile_pool(name="ps_pv", bufs=2, space="PSUM") as ps_pv,
        tc.tile_pool(name="ps1", bufs=2, space="PSUM") as ps1,
        tc.tile_pool(name="ebuf", bufs=6) as ebuf,
        tc.tile_pool(name="rbuf", bufs=4) as rbuf,
        tc.tile_pool(name="obuf", bufs=4) as obuf,
    ):

        def a_pieces(ci):
            """K and V projection pieces for t-chunk ci (8 pieces)."""
            ps = []
            cs = slice(ci * 512, ci * 512 + 512)

            def k_piece(u, cs=cs, ci=ci):
                kps = ps1.tile([128, 512], F32, tag="ps1", name="kps")
                for eb in range(4):
                    nc.tensor.matmul(
                        kps[:],
                        wk_sb[:, :, eb, u, :],
                        x8_sb[:, :, eb, cs],
                        start=(eb == 0),
                        stop=(eb == 3),
                        perf_mode=DR,
                    )
                m, di = u // 2, u % 2
                # bias+scale on DVE (Act is the exp engine)
                nc.vector.tensor_scalar(
                    kt8[:, di, m, cs],
                    kps[:],
                    ISQ / WK_SCALE,
                    bk_sb[:, u : u + 1],
                    Mult,
                    Add,
                )

            def v_piece(tbl, ci=ci):
                vps = ps1.tile([128, 512], F32, tag="ps1", name="vps")
                tb = 4 * ci + tbl
                tbs = slice(tb * 128, tb * 128 + 128)
                # compensated fp8 DoubleRow: 64*V = x8(wvh+wvl) + x8lo*wvh
                # (the 64x scale cancels in the softmax normalization)
                terms = [(x8_sb, wvh_sb), (x8_sb, wvl_sb), (x8l_sb, wvh_sb)]
                for eb in range(4):
                    for ti, (xs, ws) in enumerate(terms):
                        nc.tensor.matmul(
                            vps[:],
                            xs[:, :, eb, tbs],
                            ws[:, :, eb, :],
                            start=(eb == 0 and ti == 0),
                            stop=(eb == 3 and ti == 2),
                            perf_mode=DR,
                        )
                nc.scalar.copy(
                    v_ones[:, tb, :].rearrange("p (h c) -> p h c", c=65)[:, :, 0:64],
                    vps[:].rearrange("p (h c) -> p h c", c=64),
                )
                # fp8 hi + residual copies for the DoubleRow PV on far pairs
                h_sl = v8h[:, tb // 2, tb % 2, :].rearrange(
                    "p (h c) -> p h c", c=96)[:, :, 0:64]
                nc.vector.tensor_scalar_mul(
                    h_sl, vps[:].rearrange("p (h c) -> p h c", c=64), 1.0)
                nc.vector.tensor_tensor(
                    v8l[:, tb // 2, tb % 2, :].rearrange(
                        "p (h c) -> p h c", c=96)[:, :, 0:64],
                    vps[:].rearrange("p (h c) -> p h c", c=64),
                    h_sl,
                    mybir.AluOpType.subtract,
                )

            def kshift_piece(cs=cs):
                # matmul operands may only start at partition 0/32/64; shift
                # the upper head-groups down so S reads at bases {0, 32}
                nc.sync.dma_start(kt8b[:, :, :, cs], kt8[64:128, :, :, cs])

            for u in range(4):
                ps.append(lambda u=u: k_piece(u))
            ps.append(kshift_piece)
            for tbl in range(4):
                ps.append(lambda tbl=tbl: v_piece(tbl))
            return ps

        def b_head_piece(ci, l, filler=None):
            """Attention for local head l (=4m+a) over query chunk ci.

            `filler` (if given) is emitted late in the pair loop, where PE
            otherwise stalls on sps buffers waiting for Act to drain exps.
            """
            m, a = l // 4, l % 4
            src = kt8 if a < 2 else kt8b
            pr = slice(32 * (a % 2), 32 * (a % 2) + 32)
            cs = slice(ci * 512, ci * 512 + 512)
            njb = 4 * ci + 4
            pv = ps_pv.tile([96, 512], F32, tag="pv", name="pv")
            r_row = rbuf.tile([1, 512], F32, tag="rr", name="r_row")
            r_bc = rbuf.tile([64, 512], F32, tag="rb", name="r_bc")
            po2 = slice(64 * (l % 2), 64 * (l % 2) + 64)
            pl = l // 2

            def norm_half(qa, qb):
                # normalize finished queries while PE continues; the ones
                # columns hold 4.0 (=64/16) so o_bf comes out as 16*o
                # (wpb carries the 1/16)
                qo = slice(ci * 512 + qa, ci * 512 + qb)
                nc.vector.reciprocal(r_row[:, qa:qb], pv[64:65, qa:qb])
                nc.gpsimd.partition_broadcast(r_bc[:, qa:qb], r_row[:, qa:qb])
                nc.vector.tensor_tensor(
                    o_bf[po2, pl, qo], pv[0:64, qa:qb], r_bc[:, qa:qb], Mult
                )

            fill_at = max(njb // 2 - 2, 0)
            for jbp in range(njb // 2):
                if jbp == fill_at and filler is not None:
                    filler()
                # for the 2nd diagonal-crossing pair, queries [0, 256) are
                # entirely masked -- skip them on PE and Act
                q0 = 256 * (jbp - 2 * ci) if jbp >= 2 * ci else 0
                qs = slice(ci * 512 + q0, ci * 512 + 512)
                sps = ps_s.tile([128, 2, 512], F32, tag="s", name="sps")
                for half in (0, 1):
                    jb = 2 * jbp + half
                    nc.tensor.matmul(
                        sps[:, half, q0:512],
                        src[pr, :, m, jb * 128 : (jb + 1) * 128],
                        src[pr, :, m, qs],
                        start=True,
                        stop=True,
                        perf_mode=DR,
                    )
                if jbp < 2 * ci:
                    # far (never-masked) pair: fp8 eps + DoubleRow PV over
                    # the block pair, V error compensated by an fp8 residual
                    # term -- 2x fewer PE cycles than the bf16 path
                    ep8 = ebuf.tile([128, 2, 512], F8, tag="e8", name="ep8")
                    if jbp % 3 == 1:
                        # Schraudolph fast-exp on DVE: fp8 bits via
                        # round-to-nearest saturating f32->uint8 convert
                        nc.vector.tensor_scalar(
                            ep8[:].bitcast(U8), sps[:], FE_A, FE_B, Mult, Add)
                    else:
                        nc.scalar.activation(ep8[:], sps[:], Exp)
                    for vi, vt in enumerate((v8h, v8l)):
                        nc.tensor.matmul(
                            pv[:],
                            vt[:, jbp, :, 96 * l : 96 * l + 96],
                            ep8[:],
                            start=(jbp == 0 and vi == 0),
                            stop=False,
                            perf_mode=DR,
                            skip_group_check=True,
                        )
                    continue
                eps = ebuf.tile([128, 2, 512], BF16, tag="e", name="eps")
                nc.scalar.activation(eps[:, :, q0:512], sps[:, :, q0:512], Exp)
                # diagonal-crossing pair: zero masked entries (j > q) on
                # the idle GPSIMD engine instead of a PE mask matmul
                epm = ebuf.tile([128, 2, 512], BF16, tag="em", name="epm")
                for half in (0, 1):
                    oi = 2 * jbp + half - 4 * ci
                    nc.gpsimd.affine_select(
                        epm[:, half, q0:512],
                        eps[:, half, q0:512],
                        pattern=[[1, 512 - q0]],
                        compare_op=mybir.AluOpType.is_ge,
                        fill=0.0,
                        base=q0 - 128 * oi,
                        channel_multiplier=-1,
                    )
                eps = epm
                for half in (0, 1):
                    jb = 2 * jbp + half
                    nc.tensor.matmul(
                        pv[0:65, q0:512],
                        v_ones[:, jb, 65 * l : 65 * l + 65],
                        eps[:, half, q0:512],
                        start=(jb == 0 and ci == 0),
                        stop=(jb == njb - 1) or (jb == 4 * ci + 1),
                        skip_group_check=True,
                    )
                if jbp == 2 * ci:
                    norm_half(0, 256)
            norm_half(256, 512)

        def proj_pieces(ci):
            """Output projection pieces for t-chunk ci (8 pieces)."""
            ps = []

            def p_piece(tbl, nch, ci=ci):
                tb = 4 * ci + tbl
                tbs = slice(tb * 128, tb * 128 + 128)
                ns = slice(nch * 512, nch * 512 + 512)
                ops_ = ps1.tile([128, 512], F32, tag="ps1", name="ops")
                for blk in range(4):
                    nc.tensor.matmul(
                        ops_[:],
                        o_bf[:, blk, tbs],
                        wpb_sb[:, blk, ns],
                        start=(blk == 0),
                        stop=(blk == 3),
                    )
                ob = obuf.tile([128, 512], BF16, tag="ob", name="ob")
                nc.vector.tensor_scalar_mul(ob[:], ops_[:], 1.0)
                nc.sync.dma_start(
                    out[tb * 128 : (tb + 1) * 128, nch * 512 : (nch + 1) * 512],
                    ob[:],
                )

            for tbl in range(4):
                for nch in range(2):
                    ps.append(lambda tbl=tbl, nch=nch: p_piece(tbl, nch))
            return ps

        for f in a_pieces(0):
            f()
        for ci in range(4):
            fillers = (a_pieces(ci + 1) if ci < 3 else []) + (
                proj_pieces(ci - 1) if ci > 0 else []
            )
            # one filler inside each head (covers the mid-head sps-buffer
            # wall); spread the rest across the BACK half of the chunk so the
            # chunk tail (normalization chains) stays covered
            nrest = max(len(fillers) - 8, 0)
            bounds = [8 + (nrest * k) // 5 for k in range(6)]
            for i in range(8):
                b_head_piece(ci, i, fillers[i] if i < len(fillers) else None)
                if i >= 3:
                    for f in fillers[bounds[i - 3] : bounds[i - 2]]:
                        f()
            for f in fillers[bounds[5] :]:
                f()
        for f in proj_pieces(3):
            f()


def _build_program(nreps: int = 1, synth: bool = False):
    nc = bacc.Bacc("TRN2", target_bir_lowering=False)
    io = _declare_io(nc, synth=synth)

    with tile.TileContext(nc) as tc:
        if synth:
            _synth_init(nc, tc, io)
        with tc.tile_pool(name="singles", bufs=1) as singles:
            g = {}
            g["kt8"] = singles.tile([128, 2, 2, T], F8, name="kt8")
            g["kt8b"] = singles.tile([64, 2, 2, T], F8, name="kt8b")
            g["v_ones"] = singles.tile([128, 16, HPC * 65], BF16, name="v_ones")
            g["v8h"] = singles.tile([128, 8, 2, HPC * 96], F8, name="v8h")
            g["v8l"] = singles.tile([128, 8, 2, HPC * 96], F8, name="v8l")
            g["o_bf"] = singles.tile([128, 4, T], BF16, name="o_bf")
            g["x8_sb"] = singles.tile([128, 2, 4, T], F8, name="x8_sb")
            g["x8l_sb"] = singles.tile([128, 2, 4, T], F8, name="x8l_sb")
            g["wk_sb"] = singles.tile([128, 2, 4, 4, 128], F8, name="wk_sb")
            g["wvh_sb"] = singles.tile([128, 2, 4, 512], F8, name="wvh_sb")
            g["wvl_sb"] = singles.tile([128, 2, 4, 512], F8, name="wvl_sb")
            g["wpb_sb"] = singles.tile([128, 4, 1024], BF16, name="wpb_sb")
            g["bk_sb"] = singles.tile([128, 4], F32, name="bk_sb")

            # pad-column init first so Pool's in-order queue never delays
            # the causal selects behind its DMA descriptor generation
            nc.gpsimd.memset(
                g["v8h"][:]
                .rearrange("p j i (h c) -> p j i h c", c=96)[:, :, :, :, 64:96],
                0.0,
            )
            nc.gpsimd.memset(
                g["v8h"][:]
                .rearrange("p j i (h c) -> p j i h c", c=96)[:, :, :, :, 64:65],
                4.0,
            )
            nc.vector.memset(
                g["v8l"][:]
                .rearrange("p j i (h c) -> p j i h c", c=96)[:, :, :, :, 64:96],
                0.0,
            )
            # K-path deps (wk, bk, x8 chunk 0) first on the sync queue so the
            # first matmul starts ASAP; bulk xb/wv/wp issue from other engine
            # queues to overlap.
            nc.sync.dma_start(g["wk_sb"][:], io["wk"][:])
            nc.scalar.dma_start(g["wvh_sb"][:], io["wvh"][:])
            nc.scalar.dma_start(g["wvl_sb"][:], io["wvl"][:])
            nc.scalar.dma_start(g["wpb_sb"][:], io["wpb"][:])
            nc.gpsimd.dma_start(g["bk_sb"][:], io["bk"][:])
            for ci in range(4):
                cs = slice(ci * 512, ci * 512 + 512)
                nc.sync.dma_start(g["x8_sb"][:, :, :, cs], io["x8"][:, :, :, cs])
                nc.gpsimd.dma_start(g["x8l_sb"][:, :, :, cs], io["x8l"][:, :, :, cs])
            nc.vector.memset(
                g["v_ones"][:]
                .rearrange("p t (h c) -> p t h c", c=65)[:, :, :, 64:65],
                4.0,
            )


            for _rep in range(nreps):
                _emit_body(nc, tc, io, g)

            if synth:
                with tc.tile_pool(name="fin", bufs=1) as fin:
                    dn = fin.tile([1, 4], F32, name="dn")
                    nc.vector.memset(dn[:], 1.0)
                    nc.sync.dma_start(io["done"][:], dn[:])

    nc.compile()
    return nc


def _get_program(nreps: int = 1, synth: bool = False):
    with _cache_lock:
        key = (nreps, synth)
        if key not in _cached_nc:
            _cached_nc[key] = _build_program(nreps, synth)
        return _cached_nc[key]


def _core_inputs(c, x, W_attn, b_attn):
    import ml_dtypes

    f8 = ml_dtypes.float8_e4m3
    b = c // 2
    h0 = HPC * (c % 2)
    c0k = D + h0 * HD
    c0v = 2 * D + h0 * HD
    xt = np.ascontiguousarray(x[b].T)  # [D, T]
    # x8[p, ei, eb, t] = xt[256*eb + 128*ei + p, t]
    x8 = np.ascontiguousarray(
        xt.reshape(4, 2, 128, T).transpose(2, 1, 0, 3).astype(f8)
    )
    # fp8 residual of x^T in the same pair layout
    x8l = np.ascontiguousarray(
        (xt.reshape(4, 2, 128, T).transpose(2, 1, 0, 3)
         - x8.astype(np.float32)).astype(f8)
    )
    # wk[p, ei, eb, u, j=32a+d] = W[256eb+128ei+p, c0k + (4m+a)*64+32di+d]*WK_SCALE
    wkc = W_attn[:, c0k : c0k + 512] * WK_SCALE  # [1024, 512]
    # k-col layout: (m, a, di, d) -> col (4m+a)*64 + 32di + d
    wkc = wkc.reshape(4, 2, 128, 2, 4, 2, 32)  # [eb, ei, p, m, a, di, d]
    wk = np.ascontiguousarray(
        wkc.transpose(2, 1, 0, 3, 5, 4, 6)  # [p, ei, eb, m, di, a, d]
        .reshape(128, 2, 4, 4, 128)
        .astype(f8)
    )
    # W_v x64 (fp8 normal range) split hi + residual, x8 e-layout
    wvc = (W_attn[:, c0v : c0v + 512] * 64.0).reshape(4, 2, 128, 512).transpose(2, 1, 0, 3)
    wvh = np.ascontiguousarray(wvc.astype(f8))
    wvl = np.ascontiguousarray((wvc - wvh.astype(np.float32)).astype(f8))
    # bk[p=32a+d, u=(m,di)] = b_attn[c0k + (4m+a)*64 + 32di + d] * ISQ
    bkc = b_attn[c0k : c0k + 512].reshape(2, 4, 2, 32)  # [m, a, di, d]
    bk = np.ascontiguousarray(
        (bkc.transpose(1, 3, 0, 2) * ISQ)  # [a, d, m, di]
        .reshape(128, 4)
        .astype(np.float32)
    )
    return {
        "x8": x8,
        "x8l": x8l,
        "wk": wk,
        "wvh": wvh,
        "wvl": wvl,
        "bk": bk,
    }


def _core_wp(c, W_proj):
    import ml_dtypes

    bf = ml_dtypes.bfloat16
    h0 = HPC * (c % 2)
    r0 = h0 * HD
    # wpb[p, blk, n] = W_proj[r0 + 128*blk + p, n] / 16  (o_bf holds 16*o)
    wpb = np.ascontiguousarray(
        (W_proj[r0 : r0 + 512, :] / 16.0)
        .reshape(4, 128, 1024)
        .transpose(1, 0, 2)
        .astype(bf)
    )
    return {"wpb": wpb}


def kernel(x, W_attn, b_attn, W_proj, b_proj, **_unused):
    x = np.asarray(x, dtype=np.float32)
    W_attn = np.asarray(W_attn, dtype=np.float32)
    b_attn = np.asarray(b_attn, dtype=np.float32)
    W_proj = np.asarray(W_proj, dtype=np.float32)
    b_proj = np.asarray(b_proj, dtype=np.float32)

    nc = _get_program()
    in_maps = []
    for c in range(NCORES):
        m = _core_inputs(c, x, W_attn, b_attn)
        m.update(_core_wp(c, W_proj))
        in_maps.append(m)

    res = run_bass_kernel_spmd(nc, in_maps, core_ids=list(range(NCORES)))

    bias_row = b_proj + b_attn[2 * D : 3 * D] @ W_proj
    out = np.empty((B, T, D), dtype=np.float32)
    for b in range(B):
        out[b] = (
            res.results[2 * b]["out"].astype(np.float32)
            + res.results[2 * b + 1]["out"].astype(np.float32)
            + bias_row
        )
    return out


# revision 6
# speedup vs baseline: 4.2869x; 1.2112x over previous
"""Trainium2 Bass kernel for nn_CausalSelfAttention_38216619000057.

Reference semantics (faithful to the source bug q = k):
    qkv = x @ W_attn + b_attn ; _, k, v = split(qkv)
    S = (K K^T) * D**-0.5  (per head, causal-masked), P = softmax(S)
    out = (P V) reshaped @ W_proj + b_proj

Sharding over 8 cores: data-parallel on B (4), tensor-parallel on heads (2
groups of 8). Core c handles batch c//2, heads 8*(c%2)..8*(c%2)+7, and
produces a partial projection output; the host sums the two partials per
batch and adds b_proj + b_v @ W_proj (the V-bias contribution commutes
through softmax because rows of P sum to 1).

Engine strategy (all four compute engines balanced at ~125us of modeled
work; the f32r baseline was Act-bound at ~228us):
  * Attention-logit path in fp8e4m3 with DoubleRow perf mode (2 moving
    rows/cycle): K projection and S^T = K K^T as before.
  * The exp of the never-masked far key-block pairs is split between the
    Activation engine (true exp -> fp8, 2/3 of pairs) and the DVE engine
    (1/3 of pairs) using a one-instruction Schraudolph fast-exp: fp8 bits
    B = round(8*log2(e)*s + 55.65) computed by tensor_scalar with uint8
    saturating output, bitcast to fp8 -- the float->uint8 convert rounds
    to nearest and clamps at 0 (deep-negative logits become +0.0).
  * PV on far pairs stays fp8-DR with the exactly-compensated V =
    fp8(V) + fp8(V - fp8(V)) pair of matmuls.
  * Near-diagonal (crossing) pairs keep true exp -> bf16 on Act, causal
    zeroing by GPSIMD affine_select, and bf16 PV matmuls.
  * The attention output `o` is stored bf16 (not compensated fp8): the
    normalization multiply on DVE writes bf16 directly, and the output
    projection runs as 4 bf16 matmuls per tile. This deletes the o8
    hi/lo copies that used to occupy Act and Pool.
  * K-projection bias+scale, the fp8 V hi-copy, and the PSUM->out copy
    moved from Act to DVE tensor_scalar ops; output DMA is bf16 (host
    upcasts and adds the bias row).
  * Softmax denominators come free via a ones-column appended to V; the
    reciprocal row is broadcast across partitions on GPSIMD.

Work is software-pipelined per 512-query chunk: the K/V projection pieces
of chunk ci+1 and the output-projection pieces of chunk ci-1 are emitted as
fillers inside and between the attention head-pieces of chunk ci (weighted
toward the back half of the chunk), so PE keeps executing through the
exp/normalization latency at head and chunk boundaries. Input DMAs issue
from three engine queues with the K-path dependencies first.
"""

import threading

import numpy as np

import concourse.bacc as bacc
import concourse.mybir as mybir
import concourse.tile as tile
from concourse.bass_utils import run_bass_kernel_spmd

B, T, D = 4, 2048, 1024
H = 16
HD = 64
NCORES = 8
HPC = 8  # heads per core
ISQ = float(D**-0.5) ** 0.5  # K is pre-scaled by sqrt(D**-0.5)
WK_SCALE = 64.0  # keeps fp8 W_k columns in e4m3 normal range
LOG2E = 1.4426950408889634
FE_A = 8.0 * LOG2E  # fast-exp slope (fp8 bits per nat)
FE_B = 55.65  # fast-exp magic bias (56 - Schraudolph shift)
F32 = mybir.dt.float32
F32R = mybir.dt.float32r
BF16 = mybir.dt.bfloat16
F8 = mybir.dt.float8e4
U8 = mybir.dt.uint8
DR = mybir.MatmulPerfMode.DoubleRow

Ident = mybir.ActivationFunctionType.Identity
Exp = mybir.ActivationFunctionType.Exp
Mult = mybir.AluOpType.mult
Add = mybir.AluOpType.add

_cache_lock = threading.Lock()
_cached_nc = {}


def _declare_io(nc, synth=False):
    kind = "Internal" if synth else "ExternalInput"
    ts = {}
    # x^T in fp8 pair layout for the DoubleRow K matmul:
    # x8[p, ei, eb, t] = x[t, 256*eb + 128*ei + p]
    ts["x8"] = nc.dram_tensor("x8", [128, 2, 4, T], F8, kind=kind)
    # fp8 residual of x^T (same pair layout) for the compensated V matmul
    ts["x8l"] = nc.dram_tensor("x8l", [128, 2, 4, T], F8, kind=kind)
    # W_k fp8 (x WK_SCALE), permuted so PSUM partitions land in kt8 layout:
    # wk[p, ei, eb, u, j] with u=(m,di), j=32a+d -> k-col (4m+a)*64+32*di+d
    ts["wk"] = nc.dram_tensor("wk", [128, 2, 4, 4, 128], F8, kind=kind)
    # W_v x 64 as fp8 hi + residual, in the x8 e-layout (e=256eb+128ei+p)
    ts["wvh"] = nc.dram_tensor("wvh", [128, 2, 4, 512], F8, kind=kind)
    ts["wvl"] = nc.dram_tensor("wvl", [128, 2, 4, 512], F8, kind=kind)
    # W_proj/16 bf16: wpb[p, blk, n] = W_proj[r0 + 128*blk + p, n] / 16
    ts["wpb"] = nc.dram_tensor("wpb", [128, 4, 1024], BF16, kind=kind)
    ts["bk"] = nc.dram_tensor("bk", [128, 4], F32, kind=kind)
    ts["out"] = nc.dram_tensor("out", [T, D], BF16, kind="Internal" if synth else "ExternalOutput")
    if synth:
        ts["done"] = nc.dram_tensor("done", [1, 4], F32, kind="ExternalOutput")
    return ts


def _synth_init(nc, tc, io):
    """Fill the Internal input tensors with benign constants on device."""
    with tc.tile_pool(name="init", bufs=1) as pool:
        zt = pool.tile([128, 8192], F32, name="init_t")
        nc.vector.memset(zt[:], 0.0)
        nc.sync.dma_start(
            io["x8"][:],
            zt[:, 0:4096].bitcast(F8).rearrange("p (i e t) -> p i e t", i=2, e=4),
        )
        nc.sync.dma_start(
            io["x8l"][:],
            zt[:, 0:4096].bitcast(F8).rearrange("p (i e t) -> p i e t", i=2, e=4),
        )
        nc.sync.dma_start(
            io["wk"][:],
            zt[:, 0:1024].bitcast(F8).rearrange("p (i e u j) -> p i e u j", i=2, e=4, u=4),
        )
        nc.sync.dma_start(
            io["wvh"][:],
            zt[:, 0:1024].bitcast(F8).rearrange("p (i e n) -> p i e n", i=2, e=4),
        )
        nc.sync.dma_start(
            io["wvl"][:],
            zt[:, 0:1024].bitcast(F8).rearrange("p (i e n) -> p i e n", i=2, e=4),
        )
        nc.sync.dma_start(
            io["wpb"][:],
            zt[:, 0:2048].bitcast(BF16).rearrange("p (b n) -> p b n", b=4),
        )
        nc.sync.dma_start(io["bk"][:], zt[:, 0:4])


def _emit_body(nc, tc, io, g):
    """One full forward pass. g holds the persistent SBUF tiles."""
    kt8, v_ones = g["kt8"], g["v_ones"]
    kt8b = g["kt8b"]
    v8h, v8l = g["v8h"], g["v8l"]
    o_bf = g["o_bf"]
    x8_sb, x8l_sb = g["x8_sb"], g["x8l_sb"]
    wk_sb, wvh_sb, wvl_sb = g["wk_sb"], g["wvh_sb"], g["wvl_sb"]
    wpb_sb = g["wpb_sb"]
    bk_sb = g["bk_sb"]
    out = io["out"]

    with (
        tc.tile_pool(name="ps_s", bufs=2, space="PSUM") as ps_s,
        tc.tile_pool(name="ps_pv", bufs=2, space="PSUM") as ps_pv,
        tc.tile_pool(name="ps1", bufs=2, space="PSUM") as ps1,
        tc.tile_pool(name="ebuf", bufs=6) as ebuf,
        tc.tile_pool(name="rbuf", bufs=4) as rbuf,
        tc.tile_pool(name="obuf", bufs=4) as obuf,
    ):

        def a_pieces(ci):
            """K and V projection pieces for t-chunk ci (8 pieces)."""
            ps = []
            cs = slice(ci * 512, ci * 512 + 512)

            def k_piece(u, cs=cs, ci=ci):
                kps = ps1.tile([128, 512], F32, tag="ps1", name="kps")
                for eb in range(4):
                    nc.tensor.matmul(
                        kps[:],
                        wk_sb[:, :, eb, u, :],
                        x8_sb[:, :, eb, cs],
                        start=(eb == 0),
                        stop=(eb == 3),
                        perf_mode=DR,
                    )
                m, di = u // 2, u % 2
                # bias+scale on Act (cheaper there than on DVE; DVE takes
                # the bulk of the far-pair fast-exps instead)
                nc.scalar.activation(
                    kt8[:, di, m, cs],
                    kps[:],
                    Ident,
                    bias=bk_sb[:, u : u + 1],
                    scale=ISQ / WK_SCALE,
                )

            def v_piece(tbl, ci=ci):
                vps = ps1.tile([128, 512], F32, tag="ps1", name="vps")
                tb = 4 * ci + tbl
                tbs = slice(tb * 128, tb * 128 + 128)
                # compensated fp8 DoubleRow: 64*V = x8(wvh+wvl) + x8lo*wvh
                # (the 64x scale cancels in the softmax normalization)
                terms = [(x8_sb, wvh_sb), (x8_sb, wvl_sb), (x8l_sb, wvh_sb)]
                for eb in range(4):
                    for ti, (xs, ws) in enumerate(terms):
                        nc.tensor.matmul(
                            vps[:],
                            xs[:, :, eb, tbs],
                            ws[:, :, eb, :],
                            start=(eb == 0 and ti == 0),
                            stop=(eb == 3 and ti == 2),
                            perf_mode=DR,
                        )
                nc.scalar.copy(
                    v_ones[:, tb, :].rearrange("p (h c) -> p h c", c=65)[:, :, 0:64],
                    vps[:].rearrange("p (h c) -> p h c", c=64),
                )
                # fp8 hi + residual copies for the DoubleRow PV on far pairs
                h_sl = v8h[:, tb // 2, tb % 2, :].rearrange(
                    "p (h c) -> p h c", c=96)[:, :, 0:64]
                nc.scalar.copy(h_sl, vps[:].rearrange("p (h c) -> p h c", c=64))
                nc.vector.tensor_tensor(
                    v8l[:, tb // 2, tb % 2, :].rearrange(
                        "p (h c) -> p h c", c=96)[:, :, 0:64],
                    vps[:].rearrange("p (h c) -> p h c", c=64),
                    h_sl,
                    mybir.AluOpType.subtract,
                )

            def kshift_piece(cs=cs):
                # matmul operands may only start at partition 0/32/64; shift
                # the upper head-groups down so S reads at bases {0, 32}
                nc.sync.dma_start(kt8b[:, :, :, cs], kt8[64:128, :, :, cs])

            for u in range(4):
                ps.append(lambda u=u: k_piece(u))
            ps.append(kshift_piece)
            for tbl in range(4):
                ps.append(lambda tbl=tbl: v_piece(tbl))
            return ps

        def b_head_piece(ci, l, filler=None):
            """Attention for local head l (=4m+a) over query chunk ci.

            `filler` (if given) is emitted late in the pair loop, where PE
            otherwise stalls on sps buffers waiting for Act to drain exps.
            """
            m, a = l // 4, l % 4
            src = kt8 if a < 2 else kt8b
            pr = slice(32 * (a % 2), 32 * (a % 2) + 32)
            cs = slice(ci * 512, ci * 512 + 512)
            njb = 4 * ci + 4
            pv = ps_pv.tile([96, 512], F32, tag="pv", name="pv")
            r_row = rbuf.tile([1, 512], F32, tag="rr", name="r_row")
            r_bc = rbuf.tile([64, 512], F32, tag="rb", name="r_bc")
            po2 = slice(64 * (l % 2), 64 * (l % 2) + 64)
            pl = l // 2

            def norm_half(qa, qb):
                # normalize finished queries while PE continues; the ones
                # columns hold 4.0 (=64/16) so o_bf comes out as 16*o
                # (wpb carries the 1/16)
                qo = slice(ci * 512 + qa, ci * 512 + qb)
                nc.vector.reciprocal(r_row[:, qa:qb], pv[64:65, qa:qb])
                nc.gpsimd.partition_broadcast(r_bc[:, qa:qb], r_row[:, qa:qb])
                nc.vector.tensor_tensor(
                    o_bf[po2, pl, qo], pv[0:64, qa:qb], r_bc[:, qa:qb], Mult
                )

            fill_at = max(njb // 2 - 2, 0)
            for jbp in range(njb // 2):
                if jbp == fill_at and filler is not None:
                    filler()
                # for the 2nd diagonal-crossing pair, queries [0, 256) are
                # entirely masked -- skip them on PE and Act
                q0 = 256 * (jbp - 2 * ci) if jbp >= 2 * ci else 0
                qs = slice(ci * 512 + q0, ci * 512 + 512)
                sps = ps_s.tile([128, 2, 512], F32, tag="s", name="sps")
                for half in (0, 1):
                    jb = 2 * jbp + half
                    nc.tensor.matmul(
                        sps[:, half, q0:512],
                        src[pr, :, m, jb * 128 : (jb + 1) * 128],
                        src[pr, :, m, qs],
                        start=True,
                        stop=True,
                        perf_mode=DR,
                    )
                if jbp < 2 * ci:
                    # far (never-masked) pair: fp8 eps + DoubleRow PV over
                    # the block pair, V error compensated by an fp8 residual
                    # term -- 2x fewer PE cycles than the bf16 path
                    ep8 = ebuf.tile([128, 2, 512], F8, tag="e8", name="ep8")
                    if jbp % 3 == 1:
                        nc.scalar.activation(ep8[:], sps[:], Exp)
                    else:
                        # Schraudolph fast-exp on DVE: fp8 bits via
                        # round-to-nearest saturating f32->uint8 convert
                        nc.vector.tensor_scalar(
                            ep8[:].bitcast(U8), sps[:], FE_A, FE_B, Mult, Add)
                    for vi, vt in enumerate((v8h, v8l)):
                        nc.tensor.matmul(
                            pv[:],
                            vt[:, jbp, :, 96 * l : 96 * l + 96],
                            ep8[:],
                            start=(jbp == 0 and vi == 0),
                            stop=False,
                            perf_mode=DR,
                            skip_group_check=True,
                        )
                    continue
                eps = ebuf.tile([128, 2, 512], BF16, tag="e", name="eps")
                nc.scalar.activation(eps[:, :, q0:512], sps[:, :, q0:512], Exp)
                # diagonal-crossing pair: zero masked entries (j > q) on
                # the idle GPSIMD engine instead of a PE mask matmul
                epm = ebuf.tile([128, 2, 512], BF16, tag="em", name="epm")
                for half in (0, 1):
                    oi = 2 * jbp + half - 4 * ci
                    nc.gpsimd.affine_select(
                        epm[:, half, q0:512],
                        eps[:, half, q0:512],
                        pattern=[[1, 512 - q0]],
                        compare_op=mybir.AluOpType.is_ge,
                        fill=0.0,
                        base=q0 - 128 * oi,
                        channel_multiplier=-1,
                    )
                eps = epm
                for half in (0, 1):
                    jb = 2 * jbp + half
                    nc.tensor.matmul(
                        pv[0:65, q0:512],
                        v_ones[:, jb, 65 * l : 65 * l + 65],
                        eps[:, half, q0:512],
                        start=(jb == 0 and ci == 0),
                        stop=(jb == njb - 1) or (jb == 4 * ci + 1),
                        skip_group_check=True,
                    )
                if jbp == 2 * ci:
                    norm_half(0, 256)
            norm_half(256, 512)

        def proj_pieces(ci):
            """Output projection pieces for t-chunk ci (8 pieces)."""
            ps = []

            def p_piece(tbl, nch, ci=ci):
                tb = 4 * ci + tbl
                tbs = slice(tb * 128, tb * 128 + 128)
                ns = slice(nch * 512, nch * 512 + 512)
                ops_ = ps1.tile([128, 512], F32, tag="ps1", name="ops")
                for blk in range(4):
                    nc.tensor.matmul(
                        ops_[:],
                        o_bf[:, blk, tbs],
                        wpb_sb[:, blk, ns],
                        start=(blk == 0),
                        stop=(blk == 3),
                    )
                ob = obuf.tile([128, 512], BF16, tag="ob", name="ob")
                nc.scalar.copy(ob[:], ops_[:])
                nc.sync.dma_start(
                    out[tb * 128 : (tb + 1) * 128, nch * 512 : (nch + 1) * 512],
                    ob[:],
                )

            for tbl in range(4):
                for nch in range(2):
                    ps.append(lambda tbl=tbl, nch=nch: p_piece(tbl, nch))
            return ps

        for f in a_pieces(0):
            f()
        for ci in range(4):
            fillers = (a_pieces(ci + 1) if ci < 3 else []) + (
                proj_pieces(ci - 1) if ci > 0 else []
            )
            # one filler inside each head (covers the mid-head sps-buffer
            # wall); spread the rest across the BACK half of the chunk so the
            # chunk tail (normalization chains) stays covered
            nrest = max(len(fillers) - 8, 0)
            bounds = [8 + (nrest * k) // 5 for k in range(6)]
            for i in range(8):
                b_head_piece(ci, i, fillers[i] if i < len(fillers) else None)
                if i >= 3:
                    for f in fillers[bounds[i - 3] : bounds[i - 2]]:
                        f()
            for f in fillers[bounds[5] :]:
                f()
        for f in proj_pieces(3):
            f()


def _build_program(nreps: int = 1, synth: bool = False):
    nc = bacc.Bacc("TRN2", target_bir_lowering=False)
    io = _declare_io(nc, synth=synth)

    with tile.TileContext(nc) as tc:
        if synth:
            _synth_init(nc, tc, io)
        with tc.tile_pool(name="singles", bufs=1) as singles:
            g = {}
            g["kt8"] = singles.tile([128, 2, 2, T], F8, name="kt8")
            g["kt8b"] = singles.tile([64, 2, 2, T], F8, name="kt8b")
            g["v_ones"] = singles.tile([128, 16, HPC * 65], BF16, name="v_ones")
            g["v8h"] = singles.tile([128, 8, 2, HPC * 96], F8, name="v8h")
            g["v8l"] = singles.tile([128, 8, 2, HPC * 96], F8, name="v8l")
            g["o_bf"] = singles.tile([128, 4, T], BF16, name="o_bf")
            g["x8_sb"] = singles.tile([128, 2, 4, T], F8, name="x8_sb")
            g["x8l_sb"] = singles.tile([128, 2, 4, T], F8, name="x8l_sb")
            g["wk_sb"] = singles.tile([128, 2, 4, 4, 128], F8, name="wk_sb")
            g["wvh_sb"] = singles.tile([128, 2, 4, 512], F8, name="wvh_sb")
            g["wvl_sb"] = singles.tile([128, 2, 4, 512], F8, name="wvl_sb")
            g["wpb_sb"] = singles.tile([128, 4, 1024], BF16, name="wpb_sb")
            g["bk_sb"] = singles.tile([128, 4], F32, name="bk_sb")

            # pad-column init first so Pool's in-order queue never delays
            # the causal selects behind its DMA descriptor generation
            nc.gpsimd.memset(
                g["v8h"][:]
                .rearrange("p j i (h c) -> p j i h c", c=96)[:, :, :, :, 64:96],
                0.0,
            )
            nc.gpsimd.memset(
                g["v8h"][:]
                .rearrange("p j i (h c) -> p j i h c", c=96)[:, :, :, :, 64:65],
                4.0,
            )
            nc.vector.memset(
                g["v8l"][:]
                .rearrange("p j i (h c) -> p j i h c", c=96)[:, :, :, :, 64:96],
                0.0,
            )
            # K-path deps (wk, bk, x8 chunk 0) first on the sync queue so the
            # first matmul starts ASAP; bulk xb/wv/wp issue from other engine
            # queues to overlap.
            nc.sync.dma_start(g["wk_sb"][:], io["wk"][:])
            nc.scalar.dma_start(g["wvh_sb"][:], io["wvh"][:])
            nc.scalar.dma_start(g["wvl_sb"][:], io["wvl"][:])
            nc.scalar.dma_start(g["wpb_sb"][:], io["wpb"][:])
            nc.gpsimd.dma_start(g["bk_sb"][:], io["bk"][:])
            for ci in range(4):
                cs = slice(ci * 512, ci * 512 + 512)
                nc.sync.dma_start(g["x8_sb"][:, :, :, cs], io["x8"][:, :, :, cs])
                nc.gpsimd.dma_start(g["x8l_sb"][:, :, :, cs], io["x8l"][:, :, :, cs])
            nc.vector.memset(
                g["v_ones"][:]
                .rearrange("p t (h c) -> p t h c", c=65)[:, :, :, 64:65],
                4.0,
            )


            for _rep in range(nreps):
                _emit_body(nc, tc, io, g)

            if synth:
                with tc.tile_pool(name="fin", bufs=1) as fin:
                    dn = fin.tile([1, 4], F32, name="dn")
                    nc.vector.memset(dn[:], 1.0)
                    nc.sync.dma_start(io["done"][:], dn[:])

    nc.compile()
    return nc


def _get_program(nreps: int = 1, synth: bool = False):
    with _cache_lock:
        key = (nreps, synth)
        if key not in _cached_nc:
            _cached_nc[key] = _build_program(nreps, synth)
        return _cached_nc[key]


def _core_inputs(c, x, W_attn, b_attn):
    import ml_dtypes

    f8 = ml_dtypes.float8_e4m3
    b = c // 2
    h0 = HPC * (c % 2)
    c0k = D + h0 * HD
    c0v = 2 * D + h0 * HD
    xt = np.ascontiguousarray(x[b].T)  # [D, T]
    # x8[p, ei, eb, t] = xt[256*eb + 128*ei + p, t]
    x8 = np.ascontiguousarray(
        xt.reshape(4, 2, 128, T).transpose(2, 1, 0, 3).astype(f8)
    )
    # fp8 residual of x^T in the same pair layout
    x8l = np.ascontiguousarray(
        (xt.reshape(4, 2, 128, T).transpose(2, 1, 0, 3)
         - x8.astype(np.float32)).astype(f8)
    )
    # wk[p, ei, eb, u, j=32a+d] = W[256eb+128ei+p, c0k + (4m+a)*64+32di+d]*WK_SCALE
    wkc = W_attn[:, c0k : c0k + 512] * WK_SCALE  # [1024, 512]
    # k-col layout: (m, a, di, d) -> col (4m+a)*64 + 32di + d
    wkc = wkc.reshape(4, 2, 128, 2, 4, 2, 32)  # [eb, ei, p, m, a, di, d]
    wk = np.ascontiguousarray(
        wkc.transpose(2, 1, 0, 3, 5, 4, 6)  # [p, ei, eb, m, di, a, d]
        .reshape(128, 2, 4, 4, 128)
        .astype(f8)
    )
    # W_v x64 (fp8 normal range) split hi + residual, x8 e-layout
    wvc = (W_attn[:, c0v : c0v + 512] * 64.0).reshape(4, 2, 128, 512).transpose(2, 1, 0, 3)
    wvh = np.ascontiguousarray(wvc.astype(f8))
    wvl = np.ascontiguousarray((wvc - wvh.astype(np.float32)).astype(f8))
    # bk[p=32a+d, u=(m,di)] = b_attn[c0k + (4m+a)*64 + 32di + d] * ISQ
    bkc = b_attn[c0k : c0k + 512].reshape(2, 4, 2, 32)  # [m, a, di, d]
    bk = np.ascontiguousarray(
        (bkc.transpose(1, 3, 0, 2) * ISQ)  # [a, d, m, di]
        .reshape(128, 4)
        .astype(np.float32)
    )
    return {
        "x8": x8,
        "x8l": x8l,
        "wk": wk,
        "wvh": wvh,
        "wvl": wvl,
        "bk": bk,
    }


def _core_wp(c, W_proj):
    import ml_dtypes

    bf = ml_dtypes.bfloat16
    h0 = HPC * (c % 2)
    r0 = h0 * HD
    # wpb[p, blk, n] = W_proj[r0 + 128*blk + p, n] / 16  (o_bf holds 16*o)
    wpb = np.ascontiguousarray(
        (W_proj[r0 : r0 + 512, :] / 16.0)
        .reshape(4, 128, 1024)
        .transpose(1, 0, 2)
        .astype(bf)
    )
    return {"wpb": wpb}


def kernel(x, W_attn, b_attn, W_proj, b_proj, **_unused):
    x = np.asarray(x, dtype=np.float32)
    W_attn = np.asarray(W_attn, dtype=np.float32)
    b_attn = np.asarray(b_attn, dtype=np.float32)
    W_proj = np.asarray(W_proj, dtype=np.float32)
    b_proj = np.asarray(b_proj, dtype=np.float32)

    nc = _get_program()
    in_maps = []
    for c in range(NCORES):
        m = _core_inputs(c, x, W_attn, b_attn)
        m.update(_core_wp(c, W_proj))
        in_maps.append(m)

    res = run_bass_kernel_spmd(nc, in_maps, core_ids=list(range(NCORES)))

    bias_row = b_proj + b_attn[2 * D : 3 * D] @ W_proj
    out = np.empty((B, T, D), dtype=np.float32)
    for b in range(B):
        out[b] = (
            res.results[2 * b]["out"].astype(np.float32)
            + res.results[2 * b + 1]["out"].astype(np.float32)
            + bias_row
        )
    return out
